# revision 3
# baseline (speedup 1.0000x reference)
"""GCNConv (self-loops, symmetric norm, linear, relu) on 8 TRN2 cores.

Sharding: destination nodes row-sharded across 8 cores (1250 rows each).
Each core computes the full h = x @ W (replicated phase 0).  Source rows are
split into a COMPACT range and a DENSE range:

- Compact range (first CB*128 rows): h rows are written to per-stage HBM
  buffers; per (dst-tile, stage) the deduplicated source rows are pulled with
  dma_gather and reduced with PE matmuls against host-built scatter matrices
  S[slot, dst].  Stage sentinels let gathers (and their gpsimd descriptor
  generation) start while phase 0 is still running.
- Dense range (remaining blocks): h blocks stay resident in SBUF and are
  reduced directly with dense S[src, dst] blocks -- no h HBM write, no
  gather, no descriptor generation; costs more PE + S bytes.

Per-tile partial sums accumulate in an SBUF f32 accumulator (psum spilled per
stage).  Bias + relu on DVE, f16 out rows [1250, 301] per core; the 4-head
x12 broadcast expansion is done on the host (it is pure duplication).
"""

import numpy as np
import ml_dtypes

import concourse.bacc as bacc
import concourse.mybir as mybir
import concourse.tile as tile
from concourse.bass_utils import run_bass_kernel_spmd

N_NODES = 10000
N_GENES = 978
EMBED = 301
HEADS = 4
REP = 12
N_CORES = 8
NPC = N_NODES // N_CORES          # 1250 dst rows per core
DT = 128                          # dst tile height
NT = (NPC + DT - 1) // DT         # 10 dst tiles per core
SB = 79                           # source blocks of 128 (79*128 = 10112)
SP = SB * 128                     # padded source rows
GCH = 8                           # gene chunks: 7*128 + 82
HROW = 384                        # h row stride in f16 elems (768B, 256B mult)

# --- source-range split ---------------------------------------------------
CB = 37                           # compact source blocks  [0, CB*128)
CSTG = [13, 12, 12]               # compact stage sizes (blocks), sum == CB
DB = SB - CB                      # dense source blocks    [CB*128, SP)
DSTG_N = 7                        # dense stage size (blocks)
DSTGS = [min(DSTG_N, DB - i) for i in range(0, DB, DSTG_N)]

CST_LO = np.concatenate([[0], np.cumsum(CSTG)]) * 128  # compact stage row bounds
assert CST_LO[-1] == CB * 128

F32 = mybir.dt.float32
F16 = mybir.dt.float16
I16 = mybir.dt.int16

_prog_cache: dict = {}


def _build_program(bmax: tuple):
    """bmax: per-compact-stage max slot blocks (len == len(CSTG))."""
    nsc = len(CSTG)
    cblks = list(bmax)                      # slot blocks per compact stage
    CSLOT = sum(cblks) * 128                # compact slot columns in S
    TOT = CSLOT + DB * 128                  # total S columns
    ICOLS = sum(cblks) * 8                  # idx cols per tile (i16, /16 wrap)
    GB = max(cblks)                         # gather buffer block capacity

    nc = bacc.Bacc("TRN2", target_bir_lowering=False, debug=False,
                   num_devices=N_CORES, num_swdge_queues=4)

    xT_d = nc.dram_tensor("xT", [N_GENES, SP], F16, kind="ExternalInput")
    W_d = nc.dram_tensor("Wp", [N_GENES, EMBED], F16, kind="ExternalInput")
    b_d = nc.dram_tensor("bB", [128, EMBED], F32, kind="ExternalInput")
    S_d = nc.dram_tensor("Sblk", [NT, 128, TOT], F16, kind="ExternalInput")
    ix_d = nc.dram_tensor("idxw", [128, NT, ICOLS], I16, kind="ExternalInput")
    out_d = nc.dram_tensor("out", [NPC, EMBED], F16, kind="ExternalOutput")
    # per-stage h buffers (separate tensors scope the RAW deps per stage)
    h_ds = [nc.dram_tensor(f"hbuf{s}", [CSTG[s] * 128, HROW], F16)
            for s in range(nsc)]

    with tile.TileContext(nc) as tc:
        with (
            tc.tile_pool(name="const", bufs=1) as cpool,
            tc.tile_pool(name="sdp", bufs=2) as sdpool,
            tc.tile_pool(name="xt", bufs=3) as xpool,
            tc.tile_pool(name="hc", bufs=4) as hcpool,
            tc.tile_pool(name="gsb", bufs=10) as gpool,
            tc.tile_pool(name="osb", bufs=3) as opool,
            tc.tile_pool(name="ph0", bufs=2, space="PSUM") as ppool0,
            tc.tile_pool(name="ph1", bufs=2, space="PSUM") as ppool1,
        ):
            b_sb = cpool.tile([128, EMBED], F32)
            nc.sync.dma_start(b_sb[:], b_d[:])
            w_sb = cpool.tile([128, GCH, EMBED], F16)
            for g in range(GCH):
                gw = min(128, N_GENES - g * 128)
                nc.sync.dma_start(w_sb[:gw, g, :], W_d[g * 128:g * 128 + gw, :])
            ix_sb = cpool.tile([128, NT, ICOLS], I16)
            nc.sync.dma_start(ix_sb[:], ix_d[:])
            sc_sb = cpool.tile([128, NT, CSLOT], F16)
            for t in range(NT):
                nc.sync.dma_start(sc_sb[:, t, :], S_d[t, :, :CSLOT])
            oacc = cpool.tile([128, NT, EMBED], F32)
            hd_sb = cpool.tile([128, DB, EMBED], F16)

            # dense S stage tiles (streamed; prefetched in emission order)
            sd_tiles = []
            doff = 0
            for d, nb in enumerate(DSTGS):
                sd = sdpool.tile([128, NT, DSTG_N * 128], F16, tag="sd")
                for t in range(NT):
                    nc.sync.dma_start(
                        sd[:, t, :nb * 128],
                        S_d[t, :, CSLOT + doff * 128:CSLOT + (doff + nb) * 128])
                sd_tiles.append(sd)
                doff += nb

            # ---------------- phase 0: h = x @ W ----------------
            qc = [0]
            g_tiles = {}

            def emit_gathers(s, sent):
                nb = cblks[s]
                ioff = sum(cblks[:s]) * 8
                for t in range(NT):
                    g_sb = gpool.tile([128, GB, HROW], F16, tag="g")
                    gi = nc.gpsimd.dma_gather(
                        g_sb[:, :nb, :], h_ds[s][:],
                        ix_sb[:, t, ioff:ioff + nb * 8],
                        num_idxs=nb * 128, num_idxs_reg=nb * 128,
                        elem_size=HROW, queue_num=qc[0] % 4,
                    )
                    qc[0] += 1
                    tile.add_dep_helper(gi.ins, sent.ins, reason="gather waits h")
                    g_tiles[(s, t)] = g_sb

            SG = 512
            stage_writes = [[] for _ in range(nsc)]
            stage_done = [b for b in np.cumsum(CSTG)]  # block idx ends
            eng = [0]

            for s0 in range(0, SP, SG):
                sgw = min(SG, SP - s0)
                xt = xpool.tile([128, GCH, SG], F16, tag="xt")
                for g in range(GCH):
                    gw = min(128, N_GENES - g * 128)
                    e = nc.sync if eng[0] % 2 == 0 else nc.scalar
                    eng[0] += 1
                    e.dma_start(xt[:gw, g, :sgw],
                                xT_d[g * 128:g * 128 + gw, s0:s0 + sgw])
                for sub in range(sgw // 128):
                    blk = s0 // 128 + sub
                    ph = ppool0.tile([128, EMBED], F32)
                    for g in range(GCH):
                        gw = min(128, N_GENES - g * 128)
                        nc.tensor.matmul(
                            ph[:],
                            xt[:gw, g, sub * 128:(sub + 1) * 128],
                            w_sb[:gw, g, :],
                            start=(g == 0), stop=(g == GCH - 1),
                        )
                    if blk < CB:
                        s = int(np.searchsorted(stage_done, blk, side="right"))
                        h_sb = hcpool.tile([128, EMBED], F16, tag="hcp")
                        nc.vector.tensor_copy(h_sb[:], ph[:])
                        r0 = blk * 128 - int(CST_LO[s])
                        hw = nc.scalar.dma_start(
                            h_ds[s][r0:r0 + 128, :EMBED], h_sb[:])
                        stage_writes[s].append(hw)
                        if blk == stage_done[s] - 1:
                            sent = nc.sync.nop()
                            for w in stage_writes[s]:
                                tile.add_dep_helper(sent.ins, w.ins,
                                                    reason=f"h stage {s}")
                            emit_gathers(s, sent)
                    else:
                        nc.vector.tensor_copy(hd_sb[:, blk - CB, :], ph[:])

            # ------------- compact aggregation (stage-major) -------------
            for s in range(nsc):
                nb = cblks[s]
                coff = sum(cblks[:s]) * 128
                for t in range(NT):
                    pc = ppool1.tile([128, EMBED], F32)
                    g_sb = g_tiles[(s, t)]
                    for bi in range(nb):
                        c0 = coff + bi * 128
                        nc.tensor.matmul(
                            pc[:], sc_sb[:, t, c0:c0 + 128],
                            g_sb[:, bi, :EMBED],
                            start=(bi == 0), stop=(bi == nb - 1),
                        )
                    if s == 0:
                        nc.vector.tensor_copy(oacc[:, t, :], pc[:])
                    else:
                        nc.vector.tensor_add(oacc[:, t, :], oacc[:, t, :], pc[:])

            # ------------- dense aggregation (stage-major) -------------
            doff = 0
            for d, nb in enumerate(DSTGS):
                sd = sd_tiles[d]
                for t in range(NT):
                    pd = ppool1.tile([128, EMBED], F32)
                    for bi in range(nb):
                        nc.tensor.matmul(
                            pd[:], sd[:, t, bi * 128:(bi + 1) * 128],
                            hd_sb[:, doff + bi, :],
                            start=(bi == 0), stop=(bi == nb - 1),
                        )
                    nc.vector.tensor_add(oacc[:, t, :], oacc[:, t, :], pd[:])
                doff += nb

            # ------------- bias + relu + store -------------
            for t in range(NT):
                r0 = t * DT
                nr = min(DT, NPC - r0)
                o_sm = opool.tile([128, EMBED], F32, tag="osm")
                nc.vector.tensor_add(o_sm[:], oacc[:, t, :], b_sb[:])
                nc.vector.tensor_relu(o_sm[:], o_sm[:])
                o_f16 = opool.tile([128, EMBED], F16, tag="o16")
                nc.vector.tensor_copy(o_f16[:], o_sm[:])
                nc.scalar.dma_start(out_d[r0:r0 + nr, :], o_f16[:nr, :])

    nc.compile()
    return nc


def _preprocess(x, edge_index, edge_weight, W, b):
    nsc = len(CSTG)
    src = np.concatenate([edge_index[0].astype(np.int64),
                          np.arange(N_NODES, dtype=np.int64)])
    dst = np.concatenate([edge_index[1].astype(np.int64),
                          np.arange(N_NODES, dtype=np.int64)])
    wf = np.concatenate([edge_weight.astype(np.float32),
                         np.ones(N_NODES, np.float32)])

    deg = np.bincount(dst, weights=wf.astype(np.float64),
                      minlength=N_NODES).astype(np.float32)
    dis = np.where(deg > 0, 1.0 / np.sqrt(deg), 0.0).astype(np.float32)
    norm = (dis[src] * wf * dis[dst]).astype(np.float32)

    order = np.argsort(dst, kind="stable")
    src_s, dst_s, norm_s = src[order], dst[order], norm[order]

    core_of = dst_s // NPC
    tloc_of = (dst_s % NPC) // DT
    group = core_of * NT + tloc_of
    cnt = np.bincount(group, minlength=N_CORES * NT)
    gstart = np.zeros(N_CORES * NT + 1, np.int64)
    gstart[1:] = np.cumsum(cnt)
    dloc = (dst_s % NPC) % DT

    DLO = CB * 128  # dense range start

    # Per (core, tile): dedup compact sources per stage; dense sources map
    # directly to raw block columns.
    groups = []  # (k, t, [u_s per stage], lo, hi)
    max_u = [0] * nsc
    for g in range(N_CORES * NT):
        lo, hi = gstart[g], gstart[g + 1]
        ss = src_s[lo:hi]
        us = []
        for s in range(nsc):
            sel = np.unique(ss[(ss >= CST_LO[s]) & (ss < CST_LO[s + 1])])
            us.append(sel)
            max_u[s] = max(max_u[s], len(sel))
        groups.append((g // NT, g % NT, us, lo, hi))
    bmax = tuple(int((m + 127) // 128) for m in max_u)
    cblks = list(bmax)
    CSLOT = sum(cblks) * 128
    TOT = CSLOT + DB * 128
    ICOLS = sum(cblks) * 8

    S_f32 = np.zeros((N_CORES, NT, 128, TOT), np.float32)
    idx_arr = np.zeros((N_CORES, NT, sum(cblks) * 128), np.int16)
    for k, t, us, lo, hi in groups:
        ss = src_s[lo:hi]
        dd = dloc[lo:hi]
        nn = norm_s[lo:hi]
        # compact: slot = stage_off + rank in u_s
        slot = np.full(hi - lo, -1, np.int64)
        soff = 0
        for s in range(nsc):
            m = (ss >= CST_LO[s]) & (ss < CST_LO[s + 1])
            r = np.searchsorted(us[s], ss[m])
            slot[m] = soff + r
            idx_arr[k, t, soff:soff + len(us[s])] = (
                us[s] - CST_LO[s]).astype(np.int16)
            soff += cblks[s] * 128
        # dense: column = CSLOT + (src - DLO)
        md = ss >= DLO
        col_d = CSLOT + (ss[md] - DLO)
        np.add.at(S_f32[k, t], (col_d % 128,
                                (col_d // 128) * 128 + dd[md]), nn[md])
        mc = ~md
        np.add.at(S_f32[k, t], (slot[mc] % 128,
                                (slot[mc] // 128) * 128 + dd[mc]), nn[mc])
    S_arr = S_f32.astype(np.float16)

    # SWDGE index layout: idx i lives at (partition i%16, col i//16),
    # replicated across the 8 sixteen-partition groups.
    cols = np.arange(ICOLS)
    idx_w = np.empty((N_CORES, 128, NT, ICOLS), np.int16)
    for p in range(16):
        lane = idx_arr[:, :, cols * 16 + p]
        idx_w[:, p::16, :, :] = lane[:, None, :, :]

    xT = np.zeros((N_GENES, SP), np.float16)
    xT[:, :N_NODES] = np.ascontiguousarray(
        x.astype(np.float32).T).astype(np.float16)
    Wp = W.astype(np.float32).astype(np.float16)
    bB = np.broadcast_to(b.astype(np.float32), (128, EMBED)).copy()
    return xT, Wp, bB, S_arr, idx_w, bmax


def make_in_maps(x, edge_index, edge_weight, W, b):
    xT, Wp, bB, S_arr, idx_w, bmax = _preprocess(
        x, edge_index, edge_weight, W, b)
    in_maps = [
        {"xT": xT, "Wp": Wp, "bB": bB, "Sblk": S_arr[k], "idxw": idx_w[k]}
        for k in range(N_CORES)
    ]
    return in_maps, bmax


def get_program(bmax):
    if bmax not in _prog_cache:
        _prog_cache[bmax] = _build_program(bmax)
    return _prog_cache[bmax]


def kernel(x, edge_index, edge_weight, W, b):
    x = np.asarray(x)
    edge_index = np.asarray(edge_index)
    edge_weight = np.asarray(edge_weight)
    W = np.asarray(W)
    b = np.asarray(b)

    in_maps, bmax = make_in_maps(x, edge_index, edge_weight, W, b)
    nc = get_program(bmax)
    res = run_bass_kernel_spmd(nc, in_maps, core_ids=list(range(N_CORES)))
    o = np.concatenate([res.results[k]["out"] for k in range(N_CORES)],
                       axis=0).astype(np.float32)
    out = np.empty((N_NODES, HEADS, EMBED, REP), np.float32)
    out[:] = o[:, None, :, None]
    return out


# revision 11
# speedup vs baseline: 1.1851x; 1.1851x over previous
"""GCNConv (self-loops, symmetric norm, linear, relu) on 8 TRN2 cores.

Sharding: destination nodes row-sharded across 8 cores (1250 rows each).
Each core computes the full h = x @ W (replicated phase 0).  Source rows are
split into a COMPACT range and a DENSE range:

- Compact range (first CB*128 rows, 4 stages): h rows go to per-stage HBM
  buffers; per stage ONE batched dma_gather pulls the deduplicated source
  rows for all 10 dst tiles, which are reduced with PE matmuls against
  host-built scatter matrices S[slot, dst].  Stage sentinels let gathers
  (and their serial gpsimd descriptor generation) run while phase 0 is
  still computing.
- Dense range (remaining 42 blocks): h blocks stay resident in SBUF and are
  reduced directly with dense S[src, dst] blocks -- no h HBM write, no
  gather, no descriptor generation; costs more PE + S bytes (balances the
  DMA-vs-PE roofline).

Per-tile partials accumulate in an SBUF f32 accumulator (psum spilled per
stage).  Dense-agg stages are interleaved between compact-agg stages so PE
work covers the gather descgen of the next compact stage.  Bias + relu on
DVE, f16 out rows [1250, 301]; the 4-head x12 broadcast expansion is pure
duplication and is done on the host.

Emission-order rule: a pool-slot reuse (ring buffer) is only emitted after
the evicted slot's readers have been emitted, so Tile sees every WAR edge.
"""

import numpy as np

import concourse.bacc as bacc
import concourse.mybir as mybir
import concourse.tile as tile
from concourse.bass_utils import run_bass_kernel_spmd

N_NODES = 10000
N_GENES = 978
EMBED = 301
HEADS = 4
REP = 12
N_CORES = 8
NPC = N_NODES // N_CORES          # 1250 dst rows per core
DT = 128                          # dst tile height
NT = (NPC + DT - 1) // DT         # 10 dst tiles per core
SB = 79                           # source blocks of 128 (79*128 = 10112)
SP = SB * 128
GCH = 8                           # gene chunks: 7*128 + 82
HROW = 384                        # h row stride in f16 elems (768B)

CB = 37                           # compact source blocks  [0, CB*128)
CSTG = [10, 9, 9, 9]              # compact stage sizes (blocks)
DB = SB - CB                      # dense source blocks
DSTGS = [6] * 7                   # dense stage sizes (blocks)
assert sum(CSTG) == CB and sum(DSTGS) == DB

CST_LO = np.concatenate([[0], np.cumsum(CSTG)]) * 128
NSC = len(CSTG)

F32 = mybir.dt.float32
F16 = mybir.dt.float16
I16 = mybir.dt.int16

_prog_cache: dict = {}


def _build_program(bmax: tuple):
    """bmax: per-compact-stage max slot blocks (len == NSC)."""
    cblks = list(bmax)
    CSLOT = sum(cblks) * 128
    TOT = CSLOT + DB * 128
    ICOLS = sum(cblks) * NT * 8       # idx cols (i16), stage-major

    nc = bacc.Bacc("TRN2", target_bir_lowering=False, debug=False,
                   num_devices=N_CORES, num_swdge_queues=4)

    xT_d = nc.dram_tensor("xT", [N_GENES, SP], F16, kind="ExternalInput")
    W_d = nc.dram_tensor("Wp", [N_GENES, EMBED], F16, kind="ExternalInput")
    b_d = nc.dram_tensor("bB", [128, EMBED], F32, kind="ExternalInput")
    S_d = nc.dram_tensor("Sblk", [NT, 128, TOT], F16, kind="ExternalInput")
    ix_d = nc.dram_tensor("idxw", [128, ICOLS], I16, kind="ExternalInput")
    out_d = nc.dram_tensor("out", [NPC, EMBED], F16, kind="ExternalOutput")
    h_ds = [nc.dram_tensor(f"hbuf{s}", [CSTG[s] * 128, HROW], F16)
            for s in range(NSC)]

    GA = max(cblks[0::2])
    GBW = max(cblks[1::2])
    SDB = max(DSTGS)

    with tile.TileContext(nc) as tc:
        with (
            tc.tile_pool(name="const", bufs=1) as cpool,
            tc.tile_pool(name="scpA", bufs=1) as scpoolA,
            tc.tile_pool(name="scpB", bufs=1) as scpoolB,
            tc.tile_pool(name="sdp", bufs=2) as sdpool,
            tc.tile_pool(name="gsbA", bufs=1) as gpoolA,
            tc.tile_pool(name="gsbB", bufs=1) as gpoolB,
            tc.tile_pool(name="xt", bufs=2) as xpool,
            tc.tile_pool(name="hc", bufs=4) as hcpool,
            tc.tile_pool(name="osb", bufs=2) as opool,
            tc.tile_pool(name="ph0", bufs=2, space="PSUM") as ppool0,
            tc.tile_pool(name="ph1", bufs=2, space="PSUM") as ppool1,
        ):
            b_sb = cpool.tile([128, EMBED], F32)
            nc.sync.dma_start(b_sb[:], b_d[:])
            w_sb = cpool.tile([128, GCH, EMBED], F16)
            for g in range(GCH):
                gw = min(128, N_GENES - g * 128)
                nc.sync.dma_start(w_sb[:gw, g, :], W_d[g * 128:g * 128 + gw, :])
            ix_sb = cpool.tile([128, ICOLS], I16)
            nc.sync.dma_start(ix_sb[:], ix_d[:])
            oacc = cpool.tile([128, NT, EMBED], F32)
            hd_sb = cpool.tile([128, DB, EMBED], F16)

            sents = [None] * NSC
            stage_writes = [[] for _ in range(NSC)]
            stage_end_blk = list(np.cumsum(CSTG))
            g_tiles = [None] * NSC
            sc_tiles = [None] * NSC
            eng = [0]
            qc = [0]

            GCALL = 8               # blocks per dma_gather (1024 idxs, ring cap)

            def emit_gather(s):
                nb = cblks[s]
                ioff = sum(cblks[:s]) * NT * 8
                pool, w = (gpoolA, GA) if s % 2 == 0 else (gpoolB, GBW)
                g_sb = pool.tile([128, NT * w, HROW], F16, tag="g")
                for c0 in range(0, NT * nb, GCALL):
                    cw = min(GCALL, NT * nb - c0)
                    gi = nc.gpsimd.dma_gather(
                        g_sb[:, c0:c0 + cw, :], h_ds[s][:],
                        ix_sb[:, ioff + c0 * 8:ioff + (c0 + cw) * 8],
                        num_idxs=cw * 128, num_idxs_reg=cw * 128,
                        elem_size=HROW, queue_num=qc[0] % 4,
                    )
                    qc[0] += 1
                    tile.add_dep_helper(gi.ins, sents[s].ins,
                                        reason="gather waits h")
                g_tiles[s] = g_sb

            def emit_sc_load(s):
                nb = cblks[s]
                coff = sum(cblks[:s]) * 128
                pool, w = (scpoolA, GA) if s % 2 == 0 else (scpoolB, GBW)
                sc = pool.tile([128, NT, w * 128], F16, tag="sc")
                for t in range(NT):
                    nc.sync.dma_start(sc[:, t, :nb * 128],
                                      S_d[t, :, coff:coff + nb * 128])
                sc_tiles[s] = sc

            sd_tiles = [None] * len(DSTGS)

            def emit_sd_load(d):
                nb = DSTGS[d]
                doff = sum(DSTGS[:d]) * 128
                sd = sdpool.tile([128, NT, SDB * 128], F16, tag="sd")
                for t in range(NT):
                    nc.sync.dma_start(
                        sd[:, t, :nb * 128],
                        S_d[t, :, CSLOT + doff:CSLOT + doff + nb * 128])
                sd_tiles[d] = sd

            def agg_compact(s):
                nb = cblks[s]
                sc = sc_tiles[s]
                g_sb = g_tiles[s]
                for t in range(NT):
                    pc = ppool1.tile([128, EMBED], F32)
                    for bi in range(nb):
                        nc.tensor.matmul(
                            pc[:], sc[:, t, bi * 128:(bi + 1) * 128],
                            g_sb[:, t * nb + bi, :EMBED],
                            start=(bi == 0), stop=(bi == nb - 1),
                        )
                    if s == 0:
                        nc.vector.tensor_copy(oacc[:, t, :], pc[:])
                    else:
                        nc.vector.tensor_add(oacc[:, t, :], oacc[:, t, :],
                                             pc[:])

            def agg_dense(d):
                nb = DSTGS[d]
                doff = sum(DSTGS[:d])
                sd = sd_tiles[d]
                for t in range(NT):
                    pd = ppool1.tile([128, EMBED], F32)
                    for bi in range(nb):
                        nc.tensor.matmul(
                            pd[:], sd[:, t, bi * 128:(bi + 1) * 128],
                            hd_sb[:, doff + bi, :],
                            start=(bi == 0), stop=(bi == nb - 1),
                        )
                    nc.vector.tensor_add(oacc[:, t, :], oacc[:, t, :], pd[:])

            # ---------------- phase 0: h = x @ W ----------------
            SG = 512
            for s0 in range(0, SP, SG):
                sgw = min(SG, SP - s0)
                xt = xpool.tile([128, GCH, SG], F16, tag="xt")
                for g in range(GCH):
                    gw = min(128, N_GENES - g * 128)
                    e = nc.sync if eng[0] % 2 == 0 else nc.scalar
                    eng[0] += 1
                    e.dma_start(xt[:gw, g, :sgw],
                                xT_d[g * 128:g * 128 + gw, s0:s0 + sgw])
                for sub in range(sgw // 128):
                    blk = s0 // 128 + sub
                    ph = ppool0.tile([128, EMBED], F32)
                    for g in range(GCH):
                        gw = min(128, N_GENES - g * 128)
                        nc.tensor.matmul(
                            ph[:],
                            xt[:gw, g, sub * 128:(sub + 1) * 128],
                            w_sb[:gw, g, :],
                            start=(g == 0), stop=(g == GCH - 1),
                        )
                    if blk < CB:
                        s = int(np.searchsorted(stage_end_blk, blk,
                                                side="right"))
                        h_sb = hcpool.tile([128, EMBED], F16, tag="hcp")
                        nc.vector.tensor_copy(h_sb[:], ph[:])
                        r0 = blk * 128 - int(CST_LO[s])
                        hw = nc.scalar.dma_start(
                            h_ds[s][r0:r0 + 128, :EMBED], h_sb[:])
                        stage_writes[s].append(hw)
                        if blk == stage_end_blk[s] - 1:
                            sent = nc.sync.nop()
                            for w in stage_writes[s]:
                                tile.add_dep_helper(sent.ins, w.ins,
                                                    reason=f"h stage {s}")
                            sents[s] = sent
                            if s < 2:       # g pool bufs=2: s2/s3 in tail
                                emit_gather(s)
                    else:
                        nc.vector.tensor_copy(hd_sb[:, blk - CB, :], ph[:])
                    # sprinkled S loads for the early agg stages
                    if blk == 14:
                        emit_sc_load(0)
                    elif blk == 26:
                        emit_sc_load(1)

            # ---------------- aggregation tail ----------------
            emit_sd_load(0)
            agg_compact(0)
            emit_gather(2)          # reuses g slot of s0 (readers emitted)
            emit_sc_load(2)         # reuses sc slot of s0
            emit_sd_load(1)
            agg_dense(0)
            agg_dense(1)
            agg_compact(1)
            emit_gather(3)
            emit_sc_load(3)
            emit_sd_load(2)
            agg_dense(2)
            emit_sd_load(3)
            agg_dense(3)
            agg_compact(2)
            emit_sd_load(4)
            agg_dense(4)
            emit_sd_load(5)
            agg_compact(3)
            agg_dense(5)
            emit_sd_load(6)
            agg_dense(6)

            # ------------- bias + relu + store -------------
            for t in range(NT):
                r0 = t * DT
                nr = min(DT, NPC - r0)
                o_sm = opool.tile([128, EMBED], F32, tag="osm")
                nc.vector.tensor_add(o_sm[:], oacc[:, t, :], b_sb[:])
                nc.vector.tensor_relu(o_sm[:], o_sm[:])
                o_f16 = opool.tile([128, EMBED], F16, tag="o16")
                nc.vector.tensor_copy(o_f16[:], o_sm[:])
                nc.scalar.dma_start(out_d[r0:r0 + nr, :], o_f16[:nr, :])

    nc.compile()
    return nc


def _preprocess(x, edge_index, edge_weight, W, b):
    src = np.concatenate([edge_index[0].astype(np.int64),
                          np.arange(N_NODES, dtype=np.int64)])
    dst = np.concatenate([edge_index[1].astype(np.int64),
                          np.arange(N_NODES, dtype=np.int64)])
    wf = np.concatenate([edge_weight.astype(np.float32),
                         np.ones(N_NODES, np.float32)])

    deg = np.bincount(dst, weights=wf.astype(np.float64),
                      minlength=N_NODES).astype(np.float32)
    dis = np.where(deg > 0, 1.0 / np.sqrt(deg), 0.0).astype(np.float32)
    norm = (dis[src] * wf * dis[dst]).astype(np.float32)

    order = np.argsort(dst, kind="stable")
    src_s, dst_s, norm_s = src[order], dst[order], norm[order]

    core_of = dst_s // NPC
    tloc_of = (dst_s % NPC) // DT
    group = core_of * NT + tloc_of
    cnt = np.bincount(group, minlength=N_CORES * NT)
    gstart = np.zeros(N_CORES * NT + 1, np.int64)
    gstart[1:] = np.cumsum(cnt)
    dloc = (dst_s % NPC) % DT

    DLO = CB * 128

    groups = []
    max_u = [0] * NSC
    for g in range(N_CORES * NT):
        lo, hi = gstart[g], gstart[g + 1]
        ss = src_s[lo:hi]
        us = []
        for s in range(NSC):
            sel = np.unique(ss[(ss >= CST_LO[s]) & (ss < CST_LO[s + 1])])
            us.append(sel)
            max_u[s] = max(max_u[s], len(sel))
        groups.append((g // NT, g % NT, us, lo, hi))
    bmax = tuple(int((m + 127) // 128) for m in max_u)
    cblks = list(bmax)
    CSLOT = sum(cblks) * 128
    TOT = CSLOT + DB * 128

    S_f32 = np.zeros((N_CORES, NT, 128, TOT), np.float32)
    # idx, stage-major: per stage, concat over tiles of padded local indices
    idx_arr = np.zeros((N_CORES, NSC, NT, max(cblks) * 128), np.int16)
    for k, t, us, lo, hi in groups:
        ss = src_s[lo:hi]
        dd = dloc[lo:hi]
        nn = norm_s[lo:hi]
        slot = np.full(hi - lo, -1, np.int64)
        soff = 0
        for s in range(NSC):
            m = (ss >= CST_LO[s]) & (ss < CST_LO[s + 1])
            r = np.searchsorted(us[s], ss[m])
            slot[m] = soff + r
            idx_arr[k, s, t, :len(us[s])] = (us[s] - CST_LO[s]).astype(np.int16)
            soff += cblks[s] * 128
        md = ss >= DLO
        col_d = CSLOT + (ss[md] - DLO)
        np.add.at(S_f32[k, t], (col_d % 128,
                                (col_d // 128) * 128 + dd[md]), nn[md])
        mc = ~md
        np.add.at(S_f32[k, t], (slot[mc] % 128,
                                (slot[mc] // 128) * 128 + dd[mc]), nn[mc])
    S_arr = S_f32.astype(np.float16)

    # SWDGE wrap per stage: idx i -> (partition i%16 replicated, col i//16)
    ICOLS = sum(cblks) * NT * 8
    idx_w = np.empty((N_CORES, 128, ICOLS), np.int16)
    off = 0
    for s in range(NSC):
        n = cblks[s] * NT * 128
        flat = idx_arr[:, s, :, :cblks[s] * 128].reshape(N_CORES, n)
        cols = np.arange(n // 16)
        for p in range(16):
            lane = flat[:, cols * 16 + p]
            idx_w[:, p::16, off:off + n // 16] = lane[:, None, :]
        off += n // 16

    xT = np.zeros((N_GENES, SP), np.float16)
    xT[:, :N_NODES] = np.ascontiguousarray(
        x.astype(np.float32).T).astype(np.float16)
    Wp = W.astype(np.float32).astype(np.float16)
    bB = np.broadcast_to(b.astype(np.float32), (128, EMBED)).copy()
    return xT, Wp, bB, S_arr, idx_w, bmax


def make_in_maps(x, edge_index, edge_weight, W, b):
    xT, Wp, bB, S_arr, idx_w, bmax = _preprocess(
        x, edge_index, edge_weight, W, b)
    in_maps = [
        {"xT": xT, "Wp": Wp, "bB": bB, "Sblk": S_arr[k], "idxw": idx_w[k]}
        for k in range(N_CORES)
    ]
    return in_maps, bmax


def get_program(bmax):
    if bmax not in _prog_cache:
        _prog_cache[bmax] = _build_program(bmax)
    return _prog_cache[bmax]


def kernel(x, edge_index, edge_weight, W, b):
    x = np.asarray(x)
    edge_index = np.asarray(edge_index)
    edge_weight = np.asarray(edge_weight)
    W = np.asarray(W)
    b = np.asarray(b)

    in_maps, bmax = make_in_maps(x, edge_index, edge_weight, W, b)
    nc = get_program(bmax)
    res = run_bass_kernel_spmd(nc, in_maps, core_ids=list(range(N_CORES)))
    o = np.concatenate([res.results[k]["out"] for k in range(N_CORES)],
                       axis=0).astype(np.float32)
    out = np.empty((N_NODES, HEADS, EMBED, REP), np.float32)
    out[:] = o[:, None, :, None]
    return out
